# revision 24
# baseline (speedup 1.0000x reference)
"""FERNN cell kernel for 8x Trainium2 NeuronCores (Bass/Tile).

Computation (per sample b):
    u_conv = circ_conv(u_t, w_u)                      # [CH, 64, 64]
    u_full = zero-pad to [CH, 128, 128] (top-left)
    h_shift[c,i,j] = h_prev[c,(i+dy)%128,(j+dx)%128]  # (dx,dy) = action[b]
    out = relu(u_full + circ_conv(h_shift, w_h))

Strategy: data-parallel over batch (4 samples / core).  The per-sample roll
and the circular conv padding are folded into a host-side circular halo pad
of h_prev to [137,137]; on device each sample's conv input window is read at
a *dynamic* offset (dy, dx) loaded from the action tensor, so the conv
directly produces the rolled h_conv.

v2 data path: h is stored bf16 in DRAM and gathered via a casting gpsimd
indirect DMA (bf16 HBM traffic, float32r in SBUF) so the 9-tap conv matmuls
run at the full-rate f32r PE speed while input DMA bytes are halved; the
ReLU drain writes bf16 rows that are DMAd to a bf16 output (host upcasts).
u's im2col is prepared host-side so each sample needs a single contiguous
DMA, and its matmul opens each PSUM accumulation group so the PE starts
before the first h gather lands.  Gather offsets for all (sample, quarter)
pairs are computed once up front; PSUM tiles span two banks (half-major)
so each k-block needs a single drain; the last quarter streams out per
8-row block to shorten the epilogue tail.
"""

import numpy as np

B, CIN, CH = 32, 3, 128
WIN, WORLD, K = 64, 128, 3
NCORES = 8
BLOC = B // NCORES          # samples per core
HP = WORLD + 9              # host-padded h: rows/cols -1 .. 135  -> 137
UP = WIN + 2                # host-padded u: 66 (halo for the 3x3 im2col)

_prog_cache = {}
TRACE = False
MODE = "v2"
HQ_BUFS = 6
OROW_BUFS = 4
IM_BUFS = 2
PSUM_BUFS = 8
LAST_RESULTS = None


def _round_tf32(a):
    """Round-to-nearest-even fp32 -> tf32 (10-bit mantissa), as float32."""
    a = np.ascontiguousarray(a, np.float32)
    u = a.view(np.uint32)
    lsb = (u >> np.uint32(13)) & np.uint32(1)
    r = (u + np.uint32(0x0FFF) + lsb) & np.uint32(0xFFFFE000)
    return r.view(np.float32)


def _build_program(mode="v2", repeat=1):
    import concourse.bass as bass
    import concourse.tile as tile
    from concourse import bacc, mybir

    f32 = mybir.dt.float32
    f32r = mybir.dt.float32r
    bf16 = mybir.dt.bfloat16
    i32 = mybir.dt.int32

    nc = bacc.Bacc(
        "TRN2",
        target_bir_lowering=False,
        debug=False,
        enable_asserts=False,
        num_devices=NCORES,
    )

    h = nc.dram_tensor("h", [BLOC * CH, HP * HP], bf16, kind="ExternalInput")
    uim = nc.dram_tensor("uim", [BLOC * 9 * CIN, WIN * WIN], bf16, kind="ExternalInput")
    act = nc.dram_tensor("act", [1, 2 * BLOC], i32, kind="ExternalInput")
    wh = nc.dram_tensor("wh", [CH, 9 * CH], bf16, kind="ExternalInput")
    wu = nc.dram_tensor("wu", [128, CH], bf16, kind="ExternalInput")
    out = nc.dram_tensor("out", [BLOC * CH, WORLD, WORLD], bf16, kind="ExternalOutput")

    h_ap, uim_ap, wh_ap, wu_ap, out_ap = (
        t.ap() for t in (h, uim, wh, wu, out)
    )

    reps = repeat
    CHUNK = 33 * HP + WORLD + 2          # gathered elems per partition per quarter
    QSTEP = 32 * HP                      # element step between quarter row-starts

    with tile.TileContext(nc) as tc:
        with (
            tc.tile_pool(name="consts", bufs=1) as consts,
            tc.tile_pool(name="offs", bufs=2) as offs_tmp,
            tc.tile_pool(name="offsb", bufs=BLOC + 1) as offs_pool,
            tc.tile_pool(name="hq", bufs=HQ_BUFS) as hq_pool,
            tc.tile_pool(name="im2col", bufs=IM_BUFS) as im_pool,
            tc.tile_pool(name="orow", bufs=OROW_BUFS) as out_pool,
            tc.tile_pool(name="psum", bufs=PSUM_BUFS, space="PSUM") as psum_pool,
        ):
            # action first: it heads the longest prologue chain
            # (act -> offsets -> first gather -> h matmuls)
            act_bc = consts.tile([128, 2 * BLOC], i32)
            nc.sync.dma_start(act_bc[:], bass.AP(act, 0, [[0, 128], [1, 2 * BLOC]]))
            wu_t = consts.tile([128, CH], bf16)
            nc.sync.dma_start(wu_t[:], wu_ap[:])
            # all 4 samples' im2cols live in one persistent SBUF tile,
            # zero-padded from 9*CIN=27 to 128 contraction rows so the u
            # matmuls are K=128 like the h taps: switching the stationary
            # between different-K weights stalls the PE (~150ns/switch).
            # One memset covers the pad rows of all samples; sample 0's rows
            # ride the Activation queue ahead of wh so the u matmuls (the
            # PE's first work) unblock as early as possible
            imcs = consts.tile([128, BLOC, WIN * WIN], bf16)
            wh_t = consts.tile([CH, 9 * CH], bf16)
            nc.scalar.dma_start(wh_t[:], wh_ap[:])

            # gather offsets for every (sample, quarter), computed once:
            # offs[b][ci, q] = (b*CH+ci)*HP*HP + (dy_b + 32q)*HP + dx_b
            # (per-sample iotas: the sample stride exceeds iota's int16
            # pattern-step limit, so it must ride in `base`)
            offs = []
            for b in range(BLOC):
                qbase = offs_tmp.tile([128, 4], i32, tag="qbase")
                nc.gpsimd.iota(
                    qbase[:], pattern=[[QSTEP, 4]], base=b * CH * HP * HP,
                    channel_multiplier=HP * HP,
                )
                dyx = offs_tmp.tile([128, 1], i32, tag="dyx")
                nc.vector.tensor_scalar(
                    out=dyx[:], in0=act_bc[:, 2 * b + 1 : 2 * b + 2],
                    scalar1=HP, scalar2=None, op0=mybir.AluOpType.mult,
                )
                nc.vector.tensor_tensor(
                    out=dyx[:], in0=dyx[:], in1=act_bc[:, 2 * b : 2 * b + 1],
                    op=mybir.AluOpType.add,
                )
                ob = offs_pool.tile([128, 4], i32, tag=f"offs{b}")
                nc.vector.tensor_tensor(
                    out=ob[:], in0=qbase[:],
                    in1=dyx[:].to_broadcast([128, 4]),
                    op=mybir.AluOpType.add,
                )
                offs.append(ob)

                if b == 0:
                    # issue the pipeline-fill gather immediately, before the
                    # remaining samples' offset work queues on gpsimd; rows
                    # 0..18 land first so k=0/k=1 matmuls start early
                    hq0 = hq_pool.tile([CH, 34, HP], bf16, tag="hq")
                    hq0_flat = hq0[:].rearrange("p a c -> p (a c)")
                    ioff0 = bass.IndirectOffsetOnAxis(
                        ap=offs[0][:, 0:1], axis=1
                    )
                    SPLIT = 19 * HP
                    nc.gpsimd.indirect_dma_start(
                        out=hq0_flat[:, 0:SPLIT], out_offset=None,
                        in_=h_ap, in_offset=ioff0,
                    )
                    nc.gpsimd.indirect_dma_start(
                        out=hq0_flat[:, SPLIT:CHUNK], out_offset=None,
                        in_=h_ap, in_offset=ioff0,
                        element_offset=SPLIT,
                    )

            # pad-row memset emitted after the offset ops so it doesn't
            # delay them in the DVE queue (it gates only the u matmuls);
            # the im2col DMAs then overwrite rows 0..26 of each sample
            nc.vector.memset(imcs[:].rearrange("p b c -> p (b c)"), 0.0)
            for b in range(BLOC):
                nc.scalar.dma_start(
                    imcs[0 : 9 * CIN, b],
                    uim_ap[b * 9 * CIN : (b + 1) * 9 * CIN, :],
                )

            for _rep in range(reps):
              for b in range(BLOC):
                imc = imcs[:, b]

                for q in range(4):
                    # gather the rolled+halo'd conv input window (bf16 in
                    # DRAM -> fp32 in SBUF via casting gpsimd DMA):
                    # hq[ci, a, c] = hpad[b, ci, dy + 32q + a, dx + c]
                    if _rep == 0 and b == 0 and q == 0:
                        hq = hq0
                        skip_gather = True
                    else:
                        hq = hq_pool.tile([CH, 34, HP], bf16, tag="hq")
                        skip_gather = False
                    hq_flat = hq[:].rearrange("p a c -> p (a c)")
                    ioff = bass.IndirectOffsetOnAxis(
                        ap=offs[b][:, q : q + 1], axis=1
                    )
                    if not skip_gather:
                        nc.gpsimd.indirect_dma_start(
                            out=hq_flat[:, 0:CHUNK],
                            out_offset=None,
                            in_=h_ap,
                            in_offset=ioff,
                        )

                    for k in range(4):
                        i0 = 32 * q + 8 * k
                        # short tail: the very last quarter streams out per-k
                        last_quarter = b == BLOC - 1 and q == 3
                        if k == 0 and not last_quarter:
                            orow = out_pool.tile([CH, 32, WORLD], bf16, tag="orow32")
                        if last_quarter:
                            orow_k = out_pool.tile([CH, 8, WORLD], bf16, tag="orow8")
                        # each 64-col half is its own single-bank PSUM tile
                        # and accumulation group (8 rotating banks keep the
                        # PE ahead of the drains); drains alternate engines
                        # per half
                        for half in range(2):
                            c0 = 64 * half
                            with_u = half == 0 and i0 < WIN
                            ps = psum_pool.tile([CH, 8, 64], f32)
                            if with_u:
                                nc.tensor.matmul(
                                    ps[:],
                                    wu_t[:],
                                    imc[:, i0 * 64 : (i0 + 8) * 64],
                                    start=True,
                                    stop=False,
                                )
                            for t in range(9):
                                di, dj = t // 3, t % 3
                                rhs = hq[
                                    :,
                                    8 * k + di : 8 * k + di + 8,
                                    c0 + dj : c0 + dj + 64,
                                ]
                                nc.tensor.matmul(
                                    ps[:],
                                    wh_t[:, t * CH : (t + 1) * CH],
                                    rhs,
                                    start=(t == 0 and not with_u),
                                    stop=(t == 8),
                                )
                            # fused relu drain PSUM -> bf16 SBUF out rows
                            dst = (
                                orow_k[:, :, c0 : c0 + 64]
                                if last_quarter
                                else orow[:, 8 * k : 8 * k + 8, c0 : c0 + 64]
                            )
                            if half == 0:
                                nc.scalar.activation(
                                    dst, ps[:], mybir.ActivationFunctionType.Relu
                                )
                            else:
                                nc.vector.tensor_scalar_max(dst, ps[:], 0.0)
                        if last_quarter:
                            nc.sync.dma_start(
                                out_ap[b * CH : (b + 1) * CH, i0 : i0 + 8, :],
                                orow_k[:],
                            )
                        elif k == 3:
                            nc.sync.dma_start(
                                out_ap[b * CH : (b + 1) * CH, 32 * q : 32 * q + 32, :],
                                orow[:],
                            )

    nc.compile()
    return nc


def _get_program(mode=None, repeat=1):
    mode = mode or MODE
    key = (mode, repeat)
    if key not in _prog_cache:
        _prog_cache[key] = _build_program(mode, repeat)
    return _prog_cache[key]


def prep_in_maps(u_t, h_prev, action, w_u, w_h, mode=None):
    import ml_dtypes

    # host-side circular halo pads (pure layout; all compute stays on device)
    hpad = np.pad(h_prev, ((0, 0), (0, 0), (1, 8), (1, 8)), mode="wrap")
    upad = np.pad(u_t, ((0, 0), (0, 0), (1, 1), (1, 1)), mode="wrap")

    # u im2col on host: uim[b, (di*3+dj)*3+ci, i*64+j] = upad[b, ci, di+i, dj+j]
    # (device zero-pads the contraction rows 27..127, see _build_program)
    sw = np.lib.stride_tricks.sliding_window_view(upad, (3, 3), axis=(2, 3))
    # sw: [B, CIN, 64, 64, 3, 3] -> [B, di, dj, ci, i, j]
    uim = np.ascontiguousarray(sw.transpose(0, 4, 5, 1, 2, 3)).reshape(
        B, 9 * CIN, WIN * WIN
    )

    # weight layouts for matmul lhsT (stationary [K, M]); bf16 so LDWEIGHTS
    # takes the 16-bit fast path (background weight buffer + FWL) and
    # overlaps with the streaming matmuls
    wh_l = np.ascontiguousarray(
        w_h.transpose(1, 2, 3, 0).reshape(CH, 9 * CH)
    ).astype(ml_dtypes.bfloat16)  # [ci, (di*3+dj)*128+co]
    wu_l = np.zeros((128, CH), ml_dtypes.bfloat16)
    wu_l[: 9 * CIN] = np.ascontiguousarray(
        w_u.transpose(2, 3, 1, 0).reshape(9 * CIN, CH)
    ).astype(ml_dtypes.bfloat16)  # [(di*3+dj)*3+ci, co]
    uim = uim.astype(ml_dtypes.bfloat16)

    hpad = hpad.astype(ml_dtypes.bfloat16)

    in_maps = []
    for c in range(NCORES):
        s = slice(c * BLOC, (c + 1) * BLOC)
        in_maps.append(
            {
                "h": np.ascontiguousarray(hpad[s]).reshape(BLOC * CH, HP * HP),
                "uim": np.ascontiguousarray(uim[s]).reshape(
                    BLOC * 9 * CIN, WIN * WIN
                ),
                "act": np.ascontiguousarray(action[s]).reshape(1, 2 * BLOC),
                "wh": wh_l,
                "wu": wu_l,
            }
        )
    return in_maps


def kernel(u_t, h_prev, action, w_u, w_h):
    global LAST_RESULTS
    from concourse.bass_utils import run_bass_kernel_spmd

    nc = _get_program(MODE)

    u_t = np.asarray(u_t, np.float32)
    h_prev = np.asarray(h_prev, np.float32)
    action = np.asarray(action, np.int32)
    w_u = np.asarray(w_u, np.float32)
    w_h = np.asarray(w_h, np.float32)

    in_maps = prep_in_maps(u_t, h_prev, action, w_u, w_h, MODE)

    res = run_bass_kernel_spmd(nc, in_maps, list(range(NCORES)), trace=TRACE)
    LAST_RESULTS = res
    out = np.concatenate(
        [
            np.asarray(r["out"], dtype=np.float32).reshape(BLOC, CH, WORLD, WORLD)
            for r in res.results
        ],
        axis=0,
    )
    return out



# revision 25
# speedup vs baseline: 1.0160x; 1.0160x over previous
"""FERNN cell kernel for 8x Trainium2 NeuronCores (Bass/Tile).

Computation (per sample b):
    u_conv = circ_conv(u_t, w_u)                      # [CH, 64, 64]
    u_full = zero-pad to [CH, 128, 128] (top-left)
    h_shift[c,i,j] = h_prev[c,(i+dy)%128,(j+dx)%128]  # (dx,dy) = action[b]
    out = relu(u_full + circ_conv(h_shift, w_h))

Strategy: data-parallel over batch (4 samples / core).  The per-sample roll
and the circular conv padding are folded into a host-side circular halo pad
of h_prev to [137,137]; on device each sample's conv input window is read at
a *dynamic* offset (dy, dx) loaded from the action tensor, so the conv
directly produces the rolled h_conv.

v3 data path (all-bf16 matmuls): h is stored bf16 in DRAM and gathered by
gpsimd indirect DMA into bf16 SBUF tiles; weights and u's host-side im2col
are bf16 too, so every matmul runs the 16-bit PE path (background weight
buffer + fast weight load -- fp32r weights self-load inside InstMatmult
and stall the stream ~65 cycles/matmul).  Under sustained load the PE
drops to ~2.0 GHz (P0 power state), making the 1184 N=512 matmuls/core
~303us; this kernel measures within ~4%% of that roofline.

Structure: each 64-column half of an 8-row block is one single-bank PSUM
accumulation group (8 rotating banks keep the PE ahead of the drains);
drains alternate Activation/DVE per half and assemble 32-row bf16 tiles
that are DMAd out once per quarter (the last quarter streams per 8-row
block to shorten the tail).  All four samples' im2cols live in one
persistent SBUF tile zero-padded to 128 contraction rows, so the u
matmuls are K=128 like the h taps -- switching the stationary between
different-K weights stalls the PE ~150ns.  Gather offsets for all
(sample, quarter) pairs are computed once up front.
"""

import numpy as np

B, CIN, CH = 32, 3, 128
WIN, WORLD, K = 64, 128, 3
NCORES = 8
BLOC = B // NCORES          # samples per core
HP = WORLD + 9              # host-padded h: rows/cols -1 .. 135  -> 137
UP = WIN + 2                # host-padded u: 66 (halo for the 3x3 im2col)

_prog_cache = {}
TRACE = False
MODE = "v2"
HQ_BUFS = 4
OROW_BUFS = 3
PSUM_BUFS = 8
LAST_RESULTS = None


def _build_program(mode="v2", repeat=1):
    import concourse.bass as bass
    import concourse.tile as tile
    from concourse import bacc, mybir

    f32 = mybir.dt.float32
    bf16 = mybir.dt.bfloat16
    i32 = mybir.dt.int32

    nc = bacc.Bacc(
        "TRN2",
        target_bir_lowering=False,
        debug=False,
        enable_asserts=False,
        num_devices=NCORES,
    )

    h = nc.dram_tensor("h", [BLOC * CH, HP * HP], bf16, kind="ExternalInput")
    uim = nc.dram_tensor("uim", [BLOC * 9 * CIN, WIN * WIN], bf16, kind="ExternalInput")
    act = nc.dram_tensor("act", [1, 2 * BLOC], i32, kind="ExternalInput")
    wh = nc.dram_tensor("wh", [CH, 9 * CH], bf16, kind="ExternalInput")
    wu = nc.dram_tensor("wu", [128, CH], bf16, kind="ExternalInput")
    out = nc.dram_tensor("out", [BLOC * CH, WORLD, WORLD], bf16, kind="ExternalOutput")

    h_ap, uim_ap, wh_ap, wu_ap, out_ap = (
        t.ap() for t in (h, uim, wh, wu, out)
    )

    reps = repeat
    CHUNK = 33 * HP + WORLD + 2          # gathered elems per partition per quarter
    QSTEP = 32 * HP                      # element step between quarter row-starts

    with tile.TileContext(nc) as tc:
        with (
            tc.tile_pool(name="consts", bufs=1) as consts,
            tc.tile_pool(name="offs", bufs=2) as offs_tmp,
            tc.tile_pool(name="offsb", bufs=BLOC + 1) as offs_pool,
            tc.tile_pool(name="hq", bufs=HQ_BUFS) as hq_pool,
            tc.tile_pool(name="orow", bufs=OROW_BUFS) as out_pool,
            tc.tile_pool(name="psum", bufs=PSUM_BUFS, space="PSUM") as psum_pool,
        ):
            # action first: it heads the longest prologue chain
            # (act -> offsets -> first gather -> h matmuls)
            act_bc = consts.tile([128, 2 * BLOC], i32)
            nc.sync.dma_start(act_bc[:], bass.AP(act, 0, [[0, 128], [1, 2 * BLOC]]))
            wu_t = consts.tile([128, CH], bf16)
            nc.sync.dma_start(wu_t[:], wu_ap[:])
            # all 4 samples' im2cols live in one persistent SBUF tile,
            # zero-padded from 9*CIN=27 to 128 contraction rows so the u
            # matmuls are K=128 like the h taps: switching the stationary
            # between different-K weights stalls the PE (~150ns/switch).
            # One memset covers the pad rows of all samples; sample 0's rows
            # ride the Activation queue ahead of wh so the u matmuls (the
            # PE's first work) unblock as early as possible
            imcs = consts.tile([128, BLOC, WIN * WIN], bf16)
            wh_t = consts.tile([CH, 9 * CH], bf16)
            nc.scalar.dma_start(wh_t[:], wh_ap[:])

            # gather offsets for every (sample, quarter), computed once:
            # offs[b][ci, q] = (b*CH+ci)*HP*HP + (dy_b + 32q)*HP + dx_b
            # (per-sample iotas: the sample stride exceeds iota's int16
            # pattern-step limit, so it must ride in `base`)
            offs = []
            for b in range(BLOC):
                qbase = offs_tmp.tile([128, 4], i32, tag="qbase")
                nc.gpsimd.iota(
                    qbase[:], pattern=[[QSTEP, 4]], base=b * CH * HP * HP,
                    channel_multiplier=HP * HP,
                )
                dyx = offs_tmp.tile([128, 1], i32, tag="dyx")
                nc.vector.tensor_scalar(
                    out=dyx[:], in0=act_bc[:, 2 * b + 1 : 2 * b + 2],
                    scalar1=HP, scalar2=None, op0=mybir.AluOpType.mult,
                )
                nc.vector.tensor_tensor(
                    out=dyx[:], in0=dyx[:], in1=act_bc[:, 2 * b : 2 * b + 1],
                    op=mybir.AluOpType.add,
                )
                ob = offs_pool.tile([128, 4], i32, tag=f"offs{b}")
                nc.vector.tensor_tensor(
                    out=ob[:], in0=qbase[:],
                    in1=dyx[:].to_broadcast([128, 4]),
                    op=mybir.AluOpType.add,
                )
                offs.append(ob)

                if b == 0:
                    # issue the pipeline-fill gather immediately, before the
                    # remaining samples' offset work queues on gpsimd; rows
                    # 0..18 land first so k=0/k=1 matmuls start early
                    hq0 = hq_pool.tile([CH, 34, HP], bf16, tag="hq")
                    hq0_flat = hq0[:].rearrange("p a c -> p (a c)")
                    ioff0 = bass.IndirectOffsetOnAxis(
                        ap=offs[0][:, 0:1], axis=1
                    )
                    SPLIT = 19 * HP
                    nc.gpsimd.indirect_dma_start(
                        out=hq0_flat[:, 0:SPLIT], out_offset=None,
                        in_=h_ap, in_offset=ioff0,
                    )
                    nc.gpsimd.indirect_dma_start(
                        out=hq0_flat[:, SPLIT:CHUNK], out_offset=None,
                        in_=h_ap, in_offset=ioff0,
                        element_offset=SPLIT,
                    )

            # pad-row memset emitted after the offset ops so it doesn't
            # delay them in the DVE queue (it gates only the u matmuls);
            # the im2col DMAs then overwrite rows 0..26 of each sample
            nc.vector.memset(imcs[:].rearrange("p b c -> p (b c)"), 0.0)
            for b in range(BLOC):
                nc.scalar.dma_start(
                    imcs[0 : 9 * CIN, b],
                    uim_ap[b * 9 * CIN : (b + 1) * 9 * CIN, :],
                )

            for _rep in range(reps):
              for b in range(BLOC):
                imc = imcs[:, b]

                for q in range(4):
                    # gather the rolled+halo'd conv input window (bf16 in
                    # DRAM -> fp32 in SBUF via casting gpsimd DMA):
                    # hq[ci, a, c] = hpad[b, ci, dy + 32q + a, dx + c]
                    if _rep == 0 and b == 0 and q == 0:
                        hq = hq0
                        skip_gather = True
                    else:
                        hq = hq_pool.tile([CH, 34, HP], bf16, tag="hq")
                        skip_gather = False
                    hq_flat = hq[:].rearrange("p a c -> p (a c)")
                    ioff = bass.IndirectOffsetOnAxis(
                        ap=offs[b][:, q : q + 1], axis=1
                    )
                    if not skip_gather:
                        nc.gpsimd.indirect_dma_start(
                            out=hq_flat[:, 0:CHUNK],
                            out_offset=None,
                            in_=h_ap,
                            in_offset=ioff,
                        )

                    for k in range(4):
                        i0 = 32 * q + 8 * k
                        # short tail: the very last quarter streams out per-k
                        last_quarter = b == BLOC - 1 and q == 3
                        if k == 0 and not last_quarter:
                            orow = out_pool.tile([CH, 32, WORLD], bf16, tag="orow32")
                        if last_quarter:
                            orow_k = out_pool.tile([CH, 8, WORLD], bf16, tag="orow8")
                        # each 64-col half is its own single-bank PSUM tile
                        # and accumulation group (8 rotating banks keep the
                        # PE ahead of the drains); drains alternate engines
                        # per half
                        for half in range(2):
                            c0 = 64 * half
                            with_u = half == 0 and i0 < WIN
                            ps = psum_pool.tile([CH, 8, 64], f32)
                            if with_u:
                                nc.tensor.matmul(
                                    ps[:],
                                    wu_t[:],
                                    imc[:, i0 * 64 : (i0 + 8) * 64],
                                    start=True,
                                    stop=False,
                                )
                            for t in range(9):
                                di, dj = t // 3, t % 3
                                rhs = hq[
                                    :,
                                    8 * k + di : 8 * k + di + 8,
                                    c0 + dj : c0 + dj + 64,
                                ]
                                nc.tensor.matmul(
                                    ps[:],
                                    wh_t[:, t * CH : (t + 1) * CH],
                                    rhs,
                                    start=(t == 0 and not with_u),
                                    stop=(t == 8),
                                )
                            # fused relu drain PSUM -> bf16 SBUF out rows
                            dst = (
                                orow_k[:, :, c0 : c0 + 64]
                                if last_quarter
                                else orow[:, 8 * k : 8 * k + 8, c0 : c0 + 64]
                            )
                            if half == 0:
                                nc.scalar.activation(
                                    dst, ps[:], mybir.ActivationFunctionType.Relu
                                )
                            else:
                                nc.vector.tensor_scalar_max(dst, ps[:], 0.0)
                        if last_quarter:
                            nc.sync.dma_start(
                                out_ap[b * CH : (b + 1) * CH, i0 : i0 + 8, :],
                                orow_k[:],
                            )
                        elif k == 3:
                            nc.sync.dma_start(
                                out_ap[b * CH : (b + 1) * CH, 32 * q : 32 * q + 32, :],
                                orow[:],
                            )

    nc.compile()
    return nc


def _get_program(mode=None, repeat=1):
    mode = mode or MODE
    key = (mode, repeat)
    if key not in _prog_cache:
        _prog_cache[key] = _build_program(mode, repeat)
    return _prog_cache[key]


def prep_in_maps(u_t, h_prev, action, w_u, w_h, mode=None):
    import ml_dtypes

    # host-side circular halo pads (pure layout; all compute stays on device)
    hpad = np.pad(h_prev, ((0, 0), (0, 0), (1, 8), (1, 8)), mode="wrap")
    upad = np.pad(u_t, ((0, 0), (0, 0), (1, 1), (1, 1)), mode="wrap")

    # u im2col on host: uim[b, (di*3+dj)*3+ci, i*64+j] = upad[b, ci, di+i, dj+j]
    # (device zero-pads the contraction rows 27..127, see _build_program)
    sw = np.lib.stride_tricks.sliding_window_view(upad, (3, 3), axis=(2, 3))
    # sw: [B, CIN, 64, 64, 3, 3] -> [B, di, dj, ci, i, j]
    uim = np.ascontiguousarray(sw.transpose(0, 4, 5, 1, 2, 3)).reshape(
        B, 9 * CIN, WIN * WIN
    )

    # weight layouts for matmul lhsT (stationary [K, M]); bf16 so LDWEIGHTS
    # takes the 16-bit fast path (background weight buffer + FWL) and
    # overlaps with the streaming matmuls
    wh_l = np.ascontiguousarray(
        w_h.transpose(1, 2, 3, 0).reshape(CH, 9 * CH)
    ).astype(ml_dtypes.bfloat16)  # [ci, (di*3+dj)*128+co]
    wu_l = np.zeros((128, CH), ml_dtypes.bfloat16)
    wu_l[: 9 * CIN] = np.ascontiguousarray(
        w_u.transpose(2, 3, 1, 0).reshape(9 * CIN, CH)
    ).astype(ml_dtypes.bfloat16)  # [(di*3+dj)*3+ci, co]
    uim = uim.astype(ml_dtypes.bfloat16)

    hpad = hpad.astype(ml_dtypes.bfloat16)

    in_maps = []
    for c in range(NCORES):
        s = slice(c * BLOC, (c + 1) * BLOC)
        in_maps.append(
            {
                "h": np.ascontiguousarray(hpad[s]).reshape(BLOC * CH, HP * HP),
                "uim": np.ascontiguousarray(uim[s]).reshape(
                    BLOC * 9 * CIN, WIN * WIN
                ),
                "act": np.ascontiguousarray(action[s]).reshape(1, 2 * BLOC),
                "wh": wh_l,
                "wu": wu_l,
            }
        )
    return in_maps


def kernel(u_t, h_prev, action, w_u, w_h):
    global LAST_RESULTS
    from concourse.bass_utils import run_bass_kernel_spmd

    nc = _get_program(MODE)

    u_t = np.asarray(u_t, np.float32)
    h_prev = np.asarray(h_prev, np.float32)
    action = np.asarray(action, np.int32)
    w_u = np.asarray(w_u, np.float32)
    w_h = np.asarray(w_h, np.float32)

    in_maps = prep_in_maps(u_t, h_prev, action, w_u, w_h, MODE)

    res = run_bass_kernel_spmd(nc, in_maps, list(range(NCORES)), trace=TRACE)
    LAST_RESULTS = res
    out = np.concatenate(
        [
            np.asarray(r["out"], dtype=np.float32).reshape(BLOC, CH, WORLD, WORLD)
            for r in res.results
        ],
        axis=0,
    )
    return out



# revision 26
# speedup vs baseline: 1.0218x; 1.0056x over previous
"""FERNN cell kernel for 8x Trainium2 NeuronCores (Bass/Tile).

Computation (per sample b):
    u_conv = circ_conv(u_t, w_u)                      # [CH, 64, 64]
    u_full = zero-pad to [CH, 128, 128] (top-left)
    h_shift[c,i,j] = h_prev[c,(i+dy)%128,(j+dx)%128]  # (dx,dy) = action[b]
    out = relu(u_full + circ_conv(h_shift, w_h))

Strategy: data-parallel over batch (4 samples / core).  The per-sample roll
and the circular conv padding are folded into a host-side circular halo pad
of h_prev to [137,137]; on device each sample's conv input window is read at
a *dynamic* offset (dy, dx) loaded from the action tensor, so the conv
directly produces the rolled h_conv.

v3 data path (all-bf16 matmuls): h is stored bf16 in DRAM and gathered by
gpsimd indirect DMA into bf16 SBUF tiles; weights and u's host-side im2col
are bf16 too, so every matmul runs the 16-bit PE path (background weight
buffer + fast weight load -- fp32r weights self-load inside InstMatmult
and stall the stream ~65 cycles/matmul).  Under sustained load the PE
drops to ~2.0 GHz (P0 power state), making the 1184 N=512 matmuls/core
~303us; this kernel measures within ~4%% of that roofline.

Structure: each 64-column half of an 8-row block is one single-bank PSUM
accumulation group (8 rotating banks keep the PE ahead of the drains);
drains alternate Activation/DVE per half and assemble 32-row bf16 tiles
that are DMAd out once per quarter (the last quarter streams per 8-row
block to shorten the tail).  All four samples' im2cols live in one
persistent SBUF tile zero-padded to 128 contraction rows, so the u
matmuls are K=128 like the h taps -- switching the stationary between
different-K weights stalls the PE ~150ns.  Gather offsets for all
(sample, quarter) pairs are computed once up front.
"""

import numpy as np

B, CIN, CH = 32, 3, 128
WIN, WORLD, K = 64, 128, 3
NCORES = 8
BLOC = B // NCORES          # samples per core
HP = WORLD + 9              # host-padded h: rows/cols -1 .. 135  -> 137
UP = WIN + 2                # host-padded u: 66 (halo for the 3x3 im2col)

_prog_cache = {}
TRACE = False
MODE = "v2"
HQ_BUFS = 4
OROW_BUFS = 3
PSUM_BUFS = 8
LAST_RESULTS = None


def _build_program(mode="v2", repeat=1):
    import concourse.bass as bass
    import concourse.tile as tile
    from concourse import bacc, mybir

    f32 = mybir.dt.float32
    bf16 = mybir.dt.bfloat16
    i32 = mybir.dt.int32

    nc = bacc.Bacc(
        "TRN2",
        target_bir_lowering=False,
        debug=False,
        enable_asserts=False,
        num_devices=NCORES,
    )

    h = nc.dram_tensor("h", [BLOC * CH, HP * HP], bf16, kind="ExternalInput")
    uim = nc.dram_tensor("uim", [BLOC * 9 * CIN, WIN * WIN], bf16, kind="ExternalInput")
    act = nc.dram_tensor("act", [1, 2 * BLOC], i32, kind="ExternalInput")
    wh = nc.dram_tensor("wh", [CH, 9 * CH], bf16, kind="ExternalInput")
    wu = nc.dram_tensor("wu", [128, CH], bf16, kind="ExternalInput")
    out = nc.dram_tensor("out", [BLOC * CH, WORLD, WORLD], bf16, kind="ExternalOutput")

    h_ap, uim_ap, wh_ap, wu_ap, out_ap = (
        t.ap() for t in (h, uim, wh, wu, out)
    )

    reps = repeat
    CHUNK = 33 * HP + WORLD + 2          # gathered elems per partition per quarter
    QSTEP = 32 * HP                      # element step between quarter row-starts

    with tile.TileContext(nc) as tc:
        with (
            tc.tile_pool(name="consts", bufs=1) as consts,
            tc.tile_pool(name="offs", bufs=2) as offs_tmp,
            tc.tile_pool(name="offsb", bufs=BLOC + 1) as offs_pool,
            tc.tile_pool(name="hq", bufs=HQ_BUFS) as hq_pool,
            tc.tile_pool(name="orow", bufs=OROW_BUFS) as out_pool,
            tc.tile_pool(name="psum", bufs=PSUM_BUFS, space="PSUM") as psum_pool,
        ):
            # action first: it heads the longest prologue chain
            # (act -> offsets -> first gather -> h matmuls)
            act_bc = consts.tile([128, 2 * BLOC], i32)
            nc.sync.dma_start(act_bc[:], bass.AP(act, 0, [[0, 128], [1, 2 * BLOC]]))
            wu_t = consts.tile([128, CH], bf16)
            nc.sync.dma_start(wu_t[:], wu_ap[:])
            # all 4 samples' im2cols live in one persistent SBUF tile,
            # zero-padded from 9*CIN=27 to 128 contraction rows so the u
            # matmuls are K=128 like the h taps: switching the stationary
            # between different-K weights stalls the PE (~150ns/switch).
            # One memset covers the pad rows of all samples; sample 0's rows
            # ride the Activation queue ahead of wh so the u matmuls (the
            # PE's first work) unblock as early as possible
            imcs = consts.tile([128, BLOC, WIN * WIN], bf16)
            wh_t = consts.tile([CH, 9 * CH], bf16)
            nc.scalar.dma_start(wh_t[:], wh_ap[:])

            # gather offsets for every (sample, quarter), computed once:
            # offs[b][ci, q] = (b*CH+ci)*HP*HP + (dy_b + 32q)*HP + dx_b
            # (per-sample iotas: the sample stride exceeds iota's int16
            # pattern-step limit, so it must ride in `base`)
            offs = []
            for b in range(BLOC):
                qbase = offs_tmp.tile([128, 4], i32, tag="qbase")
                nc.gpsimd.iota(
                    qbase[:], pattern=[[QSTEP, 4]], base=b * CH * HP * HP,
                    channel_multiplier=HP * HP,
                )
                dyx = offs_tmp.tile([128, 1], i32, tag="dyx")
                nc.vector.tensor_scalar(
                    out=dyx[:], in0=act_bc[:, 2 * b + 1 : 2 * b + 2],
                    scalar1=HP, scalar2=None, op0=mybir.AluOpType.mult,
                )
                nc.vector.tensor_tensor(
                    out=dyx[:], in0=dyx[:], in1=act_bc[:, 2 * b : 2 * b + 1],
                    op=mybir.AluOpType.add,
                )
                ob = offs_pool.tile([128, 4], i32, tag=f"offs{b}")
                nc.vector.tensor_tensor(
                    out=ob[:], in0=qbase[:],
                    in1=dyx[:].to_broadcast([128, 4]),
                    op=mybir.AluOpType.add,
                )
                offs.append(ob)

                if b == 0:
                    # issue the pipeline-fill gather immediately, before the
                    # remaining samples' offset work queues on gpsimd; rows
                    # 0..18 land first so k=0/k=1 matmuls start early
                    hq0 = hq_pool.tile([CH, 34, HP], bf16, tag="hq")
                    hq0_flat = hq0[:].rearrange("p a c -> p (a c)")
                    ioff0 = bass.IndirectOffsetOnAxis(
                        ap=offs[0][:, 0:1], axis=1
                    )
                    SPLIT = 19 * HP
                    nc.gpsimd.indirect_dma_start(
                        out=hq0_flat[:, 0:SPLIT], out_offset=None,
                        in_=h_ap, in_offset=ioff0,
                    )
                    nc.gpsimd.indirect_dma_start(
                        out=hq0_flat[:, SPLIT:CHUNK], out_offset=None,
                        in_=h_ap, in_offset=ioff0,
                        element_offset=SPLIT,
                    )

            # pad-row memset emitted after the offset ops so it doesn't
            # delay them in the DVE queue (it gates only the u matmuls);
            # the im2col DMAs then overwrite rows 0..26 of each sample
            nc.vector.memset(imcs[:].rearrange("p b c -> p (b c)"), 0.0)
            for b in range(BLOC):
                nc.scalar.dma_start(
                    imcs[0 : 9 * CIN, b],
                    uim_ap[b * 9 * CIN : (b + 1) * 9 * CIN, :],
                )

            for _rep in range(reps):
              for b in range(BLOC):
                imc = imcs[:, b]

                for q in range(4):
                    # gather the rolled+halo'd conv input window (bf16 in
                    # DRAM -> fp32 in SBUF via casting gpsimd DMA):
                    # hq[ci, a, c] = hpad[b, ci, dy + 32q + a, dx + c]
                    if _rep == 0 and b == 0 and q == 0:
                        hq = hq0
                        skip_gather = True
                    else:
                        hq = hq_pool.tile([CH, 34, HP], bf16, tag="hq")
                        skip_gather = False
                    hq_flat = hq[:].rearrange("p a c -> p (a c)")
                    ioff = bass.IndirectOffsetOnAxis(
                        ap=offs[b][:, q : q + 1], axis=1
                    )
                    if not skip_gather:
                        nc.gpsimd.indirect_dma_start(
                            out=hq_flat[:, 0:CHUNK],
                            out_offset=None,
                            in_=h_ap,
                            in_offset=ioff,
                        )

                    for k in range(4):
                        i0 = 32 * q + 8 * k
                        # short tail: the very last quarter streams out per-k
                        last_quarter = b == BLOC - 1 and q == 3
                        if k == 0 and not last_quarter:
                            orow = out_pool.tile([CH, 32, WORLD], bf16, tag="orow32")
                        if last_quarter:
                            orow_k = out_pool.tile([CH, 8, WORLD], bf16, tag="orow8")
                        # each 64-col half is its own single-bank PSUM tile
                        # and accumulation group (8 rotating banks keep the
                        # PE ahead of the drains); drains alternate engines
                        # per half
                        for half in range(2):
                            c0 = 64 * half
                            with_u = half == 0 and i0 < WIN
                            ps = psum_pool.tile([CH, 8, 64], f32)
                            # u matmul closes the group rather than opening
                            # it: the h taps only need the gather, so the
                            # in-order PE queue isn't gated at kernel start
                            # on the imcs memset/DMA chain
                            for t in range(9):
                                di, dj = t // 3, t % 3
                                rhs = hq[
                                    :,
                                    8 * k + di : 8 * k + di + 8,
                                    c0 + dj : c0 + dj + 64,
                                ]
                                nc.tensor.matmul(
                                    ps[:],
                                    wh_t[:, t * CH : (t + 1) * CH],
                                    rhs,
                                    start=(t == 0),
                                    stop=(t == 8 and not with_u),
                                )
                            if with_u:
                                nc.tensor.matmul(
                                    ps[:],
                                    wu_t[:],
                                    imc[:, i0 * 64 : (i0 + 8) * 64],
                                    start=False,
                                    stop=True,
                                )
                            # fused relu drain PSUM -> bf16 SBUF out rows
                            dst = (
                                orow_k[:, :, c0 : c0 + 64]
                                if last_quarter
                                else orow[:, 8 * k : 8 * k + 8, c0 : c0 + 64]
                            )
                            if half == 0:
                                nc.scalar.activation(
                                    dst, ps[:], mybir.ActivationFunctionType.Relu
                                )
                            else:
                                nc.vector.tensor_scalar_max(dst, ps[:], 0.0)
                        if last_quarter:
                            nc.sync.dma_start(
                                out_ap[b * CH : (b + 1) * CH, i0 : i0 + 8, :],
                                orow_k[:],
                            )
                        elif k == 3:
                            nc.sync.dma_start(
                                out_ap[b * CH : (b + 1) * CH, 32 * q : 32 * q + 32, :],
                                orow[:],
                            )

    nc.compile()
    return nc


def _get_program(mode=None, repeat=1):
    mode = mode or MODE
    key = (mode, repeat)
    if key not in _prog_cache:
        _prog_cache[key] = _build_program(mode, repeat)
    return _prog_cache[key]


def prep_in_maps(u_t, h_prev, action, w_u, w_h, mode=None):
    import ml_dtypes

    # host-side circular halo pads (pure layout; all compute stays on device)
    hpad = np.pad(h_prev, ((0, 0), (0, 0), (1, 8), (1, 8)), mode="wrap")
    upad = np.pad(u_t, ((0, 0), (0, 0), (1, 1), (1, 1)), mode="wrap")

    # u im2col on host: uim[b, (di*3+dj)*3+ci, i*64+j] = upad[b, ci, di+i, dj+j]
    # (device zero-pads the contraction rows 27..127, see _build_program)
    sw = np.lib.stride_tricks.sliding_window_view(upad, (3, 3), axis=(2, 3))
    # sw: [B, CIN, 64, 64, 3, 3] -> [B, di, dj, ci, i, j]
    uim = np.ascontiguousarray(sw.transpose(0, 4, 5, 1, 2, 3)).reshape(
        B, 9 * CIN, WIN * WIN
    )

    # weight layouts for matmul lhsT (stationary [K, M]); bf16 so LDWEIGHTS
    # takes the 16-bit fast path (background weight buffer + FWL) and
    # overlaps with the streaming matmuls
    wh_l = np.ascontiguousarray(
        w_h.transpose(1, 2, 3, 0).reshape(CH, 9 * CH)
    ).astype(ml_dtypes.bfloat16)  # [ci, (di*3+dj)*128+co]
    wu_l = np.zeros((128, CH), ml_dtypes.bfloat16)
    wu_l[: 9 * CIN] = np.ascontiguousarray(
        w_u.transpose(2, 3, 1, 0).reshape(9 * CIN, CH)
    ).astype(ml_dtypes.bfloat16)  # [(di*3+dj)*3+ci, co]
    uim = uim.astype(ml_dtypes.bfloat16)

    hpad = hpad.astype(ml_dtypes.bfloat16)

    in_maps = []
    for c in range(NCORES):
        s = slice(c * BLOC, (c + 1) * BLOC)
        in_maps.append(
            {
                "h": np.ascontiguousarray(hpad[s]).reshape(BLOC * CH, HP * HP),
                "uim": np.ascontiguousarray(uim[s]).reshape(
                    BLOC * 9 * CIN, WIN * WIN
                ),
                "act": np.ascontiguousarray(action[s]).reshape(1, 2 * BLOC),
                "wh": wh_l,
                "wu": wu_l,
            }
        )
    return in_maps


def kernel(u_t, h_prev, action, w_u, w_h):
    global LAST_RESULTS
    from concourse.bass_utils import run_bass_kernel_spmd

    nc = _get_program(MODE)

    u_t = np.asarray(u_t, np.float32)
    h_prev = np.asarray(h_prev, np.float32)
    action = np.asarray(action, np.int32)
    w_u = np.asarray(w_u, np.float32)
    w_h = np.asarray(w_h, np.float32)

    in_maps = prep_in_maps(u_t, h_prev, action, w_u, w_h, MODE)

    res = run_bass_kernel_spmd(nc, in_maps, list(range(NCORES)), trace=TRACE)
    LAST_RESULTS = res
    out = np.concatenate(
        [
            np.asarray(r["out"], dtype=np.float32).reshape(BLOC, CH, WORLD, WORLD)
            for r in res.results
        ],
        axis=0,
    )
    return out

